# revision 13
# baseline (speedup 1.0000x reference)
"""Trainium2 Bass kernel for a paged-attention layer (nn_AttentionLayer).

Reference computation (shapes hardcoded from the problem spec):
    x:[4,16,4096] -> qkv = x@Wqkv.T+bqkv -> heads(32,128)
    cached K/V gathered from 48-page pool via page_table[32] (pages of 128)
    full attention (no mask) over 4096 cached + 16 new positions per batch
    out = attn_out @ Wproj.T + bproj            -> [4,16,4096] fp32

Sharding: tensor-parallel over heads. 8 cores x 4 heads. Each core gets its
slice of Wqkv/Wproj/k_pages/v_pages, computes a partial TRANSPOSED output
projection [4096,64] (bf16); partials are summed on the host + bproj.

v3 design:
  - Host dedups the page table (U unique pages); V rows pre-scaled by page
    multiplicity; the per-page block carries a count column so softmax
    numerator+denominator stay exact.
  - All DRAM inputs partition-major; few huge static DMAs issued up-front,
    split across both HWDGE queues (sync+scalar) in consumption order.
  - Every hot matmul keeps a 128-column stationary operand so FWL hides
    LDWEIGHTS: scores (K pages), AV (V pages, flipped -> produces aoT
    directly), proj (Wproj feat-tiles, flipped -> produces outT; host
    un-transposes). Softmax denominators come from count-column lhsT
    matmuls (M=1); normalization is folded into the PSUM->SBUF copy of
    aoT via a gpsimd partition-broadcast of the reciprocal row.
  - QKV runs col-tiled over chunk parity (even k -> PSUM 0:64, odd ->
    64:128) with a partition-shifting DVE copy+add merge.
"""

import os
import sys

for _p in ("/opt/trn_rl_repo", "/root/.axon_site", "/root/.axon_site/_ro/trn_rl_repo"):
    if os.path.isdir(_p) and _p not in sys.path:
        sys.path.append(_p)

import numpy as np
import ml_dtypes

import concourse.bass as bass
import concourse.bacc as bacc
import concourse.mybir as mybir
import concourse.tile as tile
from concourse.masks import make_identity
from concourse.bass_utils import run_bass_kernel_spmd

P = 128
NH = 32           # total heads
NCORES = 8
NH_L = NH // NCORES   # 4 heads per core
HD = 128
B, S = 4, 16
TOK = B * S       # 64
H = 4096
KCH = H // P      # 32 contraction chunks for x@W
PPOS = 128        # page size
PGC = 2 * HD + 1  # per-(page,head) column block: K[128] | V[128] | count
SCALE = 1.0 / float(np.sqrt(np.float32(HD)))

F32 = mybir.dt.float32
BF16 = mybir.dt.bfloat16
NPDT = ml_dtypes.bfloat16

DTYPE_NAME = "bfloat16"   # for test.py's tolerance pick


def _page_groups(U, ng=8):
    """Split U cached pages into up to ng near-equal groups."""
    gs = []
    base = 0
    for i in range(ng):
        n = (U - base + (ng - 1 - i)) // (ng - i)
        if n > 0:
            gs.append((base, base + n))
        base += n
    return gs


def build_nc(U):
    """U = number of unique pages. kvu_sb slots 0..U-1 = cached pages,
    slot U = new-token block (filled on device)."""
    nc = bacc.Bacc("TRN2", target_bir_lowering=False, debug=False)

    xT = nc.dram_tensor("xT", [P, KCH, TOK], BF16, kind="ExternalInput")
    wqT = nc.dram_tensor("wqT", [P, KCH, 512], BF16, kind="ExternalInput")
    wkvT = nc.dram_tensor("wkvT", [P, KCH, 1024], BF16, kind="ExternalInput")
    bq = nc.dram_tensor("bq", [1, 512], BF16, kind="ExternalInput")
    bkv = nc.dram_tensor("bkv", [1, 1024], BF16, kind="ExternalInput")
    kvu = nc.dram_tensor("kvu", [P, U, NH_L, PGC], BF16, kind="ExternalInput")
    wprojT = nc.dram_tensor("wprojT", [P, NH_L, H], BF16, kind="ExternalInput")
    maskt = nc.dram_tensor("maskt", [TOK, TOK], F32, kind="ExternalInput")
    outT = nc.dram_tensor("outT", [P, KCH, TOK], BF16, kind="ExternalOutput")

    with tile.TileContext(nc) as tc:
        _emit(tc, nc, U, xT, wqT, wkvT, bq, bkv, kvu, wprojT, maskt, outT)
    nc.compile()
    return nc


def _emit(tc, nc, U, xT, wqT, wkvT, bq, bkv, kvu, wprojT, maskt, outT,
          dbg=None):
    U1 = U + 1
    Exp = mybir.ActivationFunctionType.Exp
    Add = mybir.AluOpType.add
    Mult = mybir.AluOpType.mult
    groups = _page_groups(U)
    nwb = (len(groups) + 1) // 2
    kchq = KCH // (2 * nwb) if nwb else KCH
    maxg = max(g1 - g0 for (g0, g1) in groups)
    gw = maxg + 1  # attnT buffer width in chunks (newtok uses the extra slot)

    with (
        tc.tile_pool(name="cbuf", bufs=1) as cb,
        tc.tile_pool(name="wpp", bufs=4) as wpp,
        tc.tile_pool(name="big", bufs=2, space="PSUM") as bigp,
        tc.tile_pool(name="scp", bufs=2, space="PSUM") as scp,
        tc.tile_pool(name="avp", bufs=1, space="PSUM") as avp,
        tc.tile_pool(name="dnp", bufs=1, space="PSUM") as dnp,
        tc.tile_pool(name="prp", bufs=2, space="PSUM") as prp,
    ):
        ctr = [0]

        def big_tile(dt=F32):
            ctr[0] += 1
            return bigp.tile([P, 512], dt, tag="big", name=f"big{ctr[0]}")

        def sc_tile():
            ctr[0] += 1
            return scp.tile([P, 512], F32, tag="sc", name=f"sc{ctr[0]}")

        def sc_tile_bf():
            ctr[0] += 1
            return scp.tile([P, 512], BF16, tag="sc", name=f"sc{ctr[0]}")

        # ---- resident SBUF tiles ----
        xT_sb = cb.tile([P, KCH, TOK], BF16, tag="xT")
        wq_sb = cb.tile([P, KCH, 512], BF16, tag="wq")
        wkv_sb = cb.tile([P, KCH, 1024], BF16, tag="wkv")
        kvu_sb = cb.tile([P, U1, NH_L, PGC], BF16, tag="kvu")
        ident = cb.tile([P, P], BF16, tag="ident")
        bq_sb = cb.tile([1, 512], BF16, tag="bq")
        bkv_sb = cb.tile([1, 1024], BF16, tag="bkv")
        ones_sb = cb.tile([1, TOK], BF16, tag="ones")
        mask_sb = cb.tile([TOK, TOK], F32, tag="mask")
        qT_sb = cb.tile([P, NH_L, TOK], BF16, tag="qT")
        aoT_sb = cb.tile([P, NH_L, TOK], BF16, tag="aoT")
        qkv_q = cb.tile([TOK, 512], BF16, tag="qkv_q")
        qkv_kv = cb.tile([TOK, 1024], BF16, tag="qkv_kv")
        attnT = cb.tile([P, NH_L, 2, gw * TOK], BF16, tag="attnT")
        hi_tmp = cb.tile([TOK, 512], F32, tag="hi")
        denr = cb.tile([1, NH_L * TOK], F32, tag="denr")
        rbc = cb.tile([P, NH_L * TOK], F32, tag="rbc")
        obT = cb.tile([P, KCH, TOK], BF16, tag="obT")

        # ---- DMA schedule: emission order == arrival order == PE
        # consumption order. wq halves first, then page groups and wkv
        # quarter-batches alternating on both queues, wproj last.
        nc.sync.dma_start(wq_sb[:, 0:16, :], wqT[:, 0:16, :])
        nc.scalar.dma_start(wq_sb[:, 16:32, :], wqT[:, 16:32, :])
        nc.sync.dma_start(xT_sb[:], xT[:])
        nc.scalar.dma_start(bq_sb[:], bq[:])
        nc.scalar.dma_start(bkv_sb[:], bkv[:])
        nc.scalar.dma_start(mask_sb[:], maskt[:])
        for gi, (g0, g1) in enumerate(groups):
            eng = nc.sync if gi % 2 == 0 else nc.scalar
            eng.dma_start(kvu_sb[:, g0:g1, :, :], kvu[:, g0:g1, :, :])
            k0 = gi * kchq
            k1 = KCH if gi == len(groups) - 1 else (gi + 1) * kchq
            if k0 < k1:
                eng.dma_start(wkv_sb[:, k0:k1, :], wkvT[:, k0:k1, :])
        wp_tiles = []
        for sidx in range(4):
            wp = wpp.tile([P, NH_L, 1024], BF16, tag="wp", name=f"wp{sidx}")
            eng = nc.sync if sidx < 2 else nc.scalar
            eng.dma_start(wp[:], wprojT[:, :, sidx * 1024:(sidx + 1) * 1024])
            wp_tiles.append(wp)

        # ---- setup ----
        make_identity(nc, ident[:])
        nc.gpsimd.memset(ones_sb[:], 1.0)
        # new-token slot U: zero K pad + V rows + count, count=1 valid rows
        nc.gpsimd.memset(kvu_sb[:, U, :, :], 0.0)
        nc.gpsimd.memset(kvu_sb[:TOK, U, :, 2 * HD:], 1.0)

        # warm the PE HAM clock gate while the first DMAs land
        ps_warm = big_tile()
        for _ in range(30):
            nc.tensor.matmul(
                ps_warm[:, :P], lhsT=ident[:], rhs=ident[:],
                start=True, stop=True,
            )

        # ---- QKV (q part): col-tiled over chunk parity ----
        ps_q = big_tile()
        for k in range(KCH):
            par = k % 2
            nc.tensor.matmul(
                ps_q[64 * par:64 * (par + 1), :],
                lhsT=xT_sb[:, k, :],
                rhs=wq_sb[:, k, :],
                start=(k < 2),
                stop=(k == KCH - 1),
                tile_position=(0, 64 * par),
            )
        nc.tensor.matmul(
            ps_q[64:128, :], lhsT=ones_sb[:], rhs=bq_sb[:],
            start=False, stop=True, tile_position=(0, 64),
        )
        nc.vector.tensor_copy(hi_tmp[:], ps_q[64:128, :])
        nc.vector.tensor_tensor(
            out=qkv_q[:], in0=ps_q[0:64, :], in1=hi_tmp[:], op=Add
        )
        for hl in range(NH_L):
            ps_t = big_tile(BF16)
            nc.tensor.transpose(
                ps_t[:, :TOK], qkv_q[:, hl * HD:(hl + 1) * HD],
                ident[:TOK, :TOK],
            )
            nc.vector.tensor_copy(qT_sb[:, hl, :], ps_t[:, :TOK])

        # ---- attention (kv-QKV batches interleaved below) ----
        # ps_av[:, hl*64:(hl+1)*64] accumulates unnormalized aoT per head
        ps_av = avp.tile([P, NH_L * TOK], F32, tag="av")
        ps_den = dnp.tile([1, NH_L * TOK], F32, tag="den")

        def attn_block(gi, g0, g1, is_new):
            par = gi % 2
            n = g1 - g0
            # scores + exp, all heads
            for hl in range(NH_L):
                ps_sc = sc_tile()
                for c in range(g0, g1):
                    nc.tensor.matmul(
                        ps_sc[:, (c - g0) * TOK:(c - g0 + 1) * TOK],
                        lhsT=kvu_sb[:, c, hl, 0:PPOS],
                        rhs=qT_sb[:, hl, :],
                        start=True, stop=True,
                    )
                if is_new:  # block-diagonal batch mask on raw scores
                    nc.vector.tensor_tensor(
                        out=ps_sc[:TOK, 0:TOK], in0=ps_sc[:TOK, 0:TOK],
                        in1=mask_sb[:], op=Add,
                    )
                nc.scalar.activation(
                    attnT[:, hl, par, 0:n * TOK], ps_sc[:, 0:n * TOK],
                    Exp, scale=SCALE,
                )
            # AV (flipped: V stationary, 128-col FWL) + denominator
            for hl in range(NH_L):
                for c in range(g0, g1):
                    a_sl = attnT[:, hl, par, (c - g0) * TOK:(c - g0 + 1) * TOK]
                    nc.tensor.matmul(
                        ps_av[:, hl * TOK:(hl + 1) * TOK],
                        lhsT=kvu_sb[:, c, hl, PPOS:PPOS + HD],
                        rhs=a_sl,
                        start=(gi == 0 and hl == 0 and c == g0),
                        stop=(is_new and hl == NH_L - 1 and c == g1 - 1),
                        skip_group_check=True,
                    )
                for c in range(g0, g1):
                    a_sl = attnT[:, hl, par, (c - g0) * TOK:(c - g0 + 1) * TOK]
                    nc.tensor.matmul(
                        ps_den[:, hl * TOK:(hl + 1) * TOK],
                        lhsT=kvu_sb[:, c, hl, 2 * HD:PGC],
                        rhs=a_sl,
                        start=(gi == 0 and hl == 0 and c == g0),
                        stop=(is_new and hl == NH_L - 1 and c == g1 - 1),
                        skip_group_check=True,
                    )

        ps_kv = [big_tile(), big_tile()]

        def kv_batch(k0, k1):
            for k in range(k0, k1):
                par = k % 2
                for j in range(2):
                    nc.tensor.matmul(
                        ps_kv[j][64 * par:64 * (par + 1), :],
                        lhsT=xT_sb[:, k, :],
                        rhs=wkv_sb[:, k, 512 * j:512 * (j + 1)],
                        start=(k < 2),
                        stop=(k == KCH - 1),
                        tile_position=(0, 64 * par),
                    )

        for gi, (g0, g1) in enumerate(groups):
            attn_block(gi, g0, g1, False)
            k0 = gi * kchq
            k1 = KCH if gi == len(groups) - 1 else (gi + 1) * kchq
            if k0 < k1:
                kv_batch(k0, k1)
        # kv bias + merge + new-token slot fill
        for j in range(2):
            nc.tensor.matmul(
                ps_kv[j][64:128, :], lhsT=ones_sb[:],
                rhs=bkv_sb[:, 512 * j:512 * (j + 1)],
                start=False, stop=True, tile_position=(0, 64),
            )
        for j in range(2):
            nc.vector.tensor_copy(hi_tmp[:], ps_kv[j][64:128, :])
            nc.vector.tensor_tensor(
                out=qkv_kv[:, 512 * j:512 * (j + 1)],
                in0=ps_kv[j][0:64, :], in1=hi_tmp[:], op=Add,
            )
        for hl in range(NH_L):
            ps_t = sc_tile_bf()
            nc.tensor.transpose(
                ps_t[:, :TOK], qkv_kv[:, hl * 256:hl * 256 + HD],
                ident[:TOK, :TOK],
            )
            nc.vector.tensor_copy(kvu_sb[:, U, hl, 0:TOK], ps_t[:, :TOK])
            nc.vector.tensor_copy(
                kvu_sb[:TOK, U, hl, PPOS:PPOS + HD],
                qkv_kv[:, hl * 256 + HD:hl * 256 + 2 * HD],
            )
        attn_block(len(groups), U, U1, True)

        # ---- normalize: recip row -> broadcast -> fold into aoT copy ----
        nc.vector.reciprocal(denr[:], ps_den[:])
        nc.gpsimd.partition_broadcast(rbc[:], denr[:])
        for hl in range(NH_L):
            nc.vector.tensor_tensor(
                out=aoT_sb[:, hl, :],
                in0=ps_av[:, hl * TOK:(hl + 1) * TOK],
                in1=rbc[:, hl * TOK:(hl + 1) * TOK],
                op=Mult,
            )

        # ---- output projection (flipped: Wproj tiles stationary) ----
        for sidx in range(4):
            wp = wp_tiles[sidx]
            ps_o = prp.tile([P, 512], F32, tag="pr", name=f"pr{sidx}")
            for fi in range(8):
                for hl in range(NH_L):
                    nc.tensor.matmul(
                        ps_o[:, fi * TOK:(fi + 1) * TOK],
                        lhsT=wp[:, hl, fi * HD:(fi + 1) * HD],
                        rhs=aoT_sb[:, hl, :],
                        start=(hl == 0), stop=(hl == NH_L - 1),
                    )
            nc.vector.tensor_copy(
                obT[:, sidx * 8:(sidx + 1) * 8, :], ps_o[:]
            )
            if sidx % 2 == 1:
                half = sidx // 2
                nc.sync.dma_start(
                    outT[:, half * 16:(half + 1) * 16, :],
                    obT[:, half * 16:(half + 1) * 16, :],
                )

        if dbg is not None:
            nc.sync.dma_start(dbg["d_qT"][:], qT_sb[:])
            nc.sync.dma_start(dbg["d_aoT"][:], aoT_sb[:])
            nc.sync.dma_start(dbg["d_denr"][:], denr[:])
            nc.sync.dma_start(dbg["d_kvuU"][:], kvu_sb[:, U, :, :])
            nc.sync.dma_start(dbg["d_qkvkv"][:], qkv_kv[:])


_NC_CACHE = {}


def _get_nc(U):
    if U not in _NC_CACHE:
        _NC_CACHE[U] = build_nc(U)
    return _NC_CACHE[U]


def _host_prep(x, Wqkv, bqkv, Wproj, k_pages, v_pages, page_table):
    """Build the 8 per-core input maps (numpy, partition-major layouts)."""
    x = np.asarray(x, np.float32)
    Wqkv = np.asarray(Wqkv, np.float32)
    bqkv = np.asarray(bqkv, np.float32)
    Wproj = np.asarray(Wproj, np.float32)
    k_pages = np.asarray(k_pages, np.float32)
    v_pages = np.asarray(v_pages, np.float32)
    pt = np.asarray(page_table, np.int64)

    upages, counts = np.unique(pt, return_counts=True)
    U = len(upages)

    xT = np.ascontiguousarray(
        x.reshape(TOK, H).T.reshape(KCH, P, TOK).transpose(1, 0, 2)
    ).astype(NPDT)

    mask = np.full((TOK, TOK), -1e30, np.float32)
    for b in range(B):
        mask[b * S:(b + 1) * S, b * S:(b + 1) * S] = 0.0

    Wq, Wk, Wv = Wqkv[:H], Wqkv[H:2 * H], Wqkv[2 * H:]
    bqf, bkf, bvf = bqkv[:H], bqkv[H:2 * H], bqkv[2 * H:]

    # gather unique pages once for all cores: [U, PPOS, NH, HD]
    ku = k_pages[upages]
    vu = v_pages[upages] * counts[:, None, None, None].astype(np.float32)

    in_maps = []
    for c in range(NCORES):
        h0 = c * NH_L
        hs = slice(h0 * HD, (h0 + NH_L) * HD)

        wqT = np.ascontiguousarray(
            Wq[hs].T.reshape(KCH, P, NH_L * HD).transpose(1, 0, 2)
        ).astype(NPDT)
        wkv_rows = np.empty((NH_L * 2 * HD, H), np.float32)
        for h in range(NH_L):
            r = slice((h0 + h) * HD, (h0 + h + 1) * HD)
            wkv_rows[h * 256:h * 256 + HD] = Wk[r]
            wkv_rows[h * 256 + HD:h * 256 + 2 * HD] = Wv[r]
        wkvT = np.ascontiguousarray(
            wkv_rows.T.reshape(KCH, P, 1024).transpose(1, 0, 2)
        ).astype(NPDT)

        bq_l = bqf[hs].reshape(1, 512).astype(NPDT)
        bkv_l = np.empty((1, 1024), np.float32)
        for h in range(NH_L):
            bkv_l[0, h * 256:h * 256 + HD] = bkf[(h0 + h) * HD:(h0 + h + 1) * HD]
            bkv_l[0, h * 256 + HD:h * 256 + 2 * HD] = (
                bvf[(h0 + h) * HD:(h0 + h + 1) * HD]
            )
        bkv_l = bkv_l.astype(NPDT)

        kvu_arr = np.zeros((P, U, NH_L, PGC), np.float32)
        # K block: [p=hd, u, h, c=pos]
        kvu_arr[:, :, :, 0:PPOS] = ku[:, :, h0:h0 + NH_L, :].transpose(3, 0, 2, 1)
        # V block: [p=pos, u, h, c=hd] (count-scaled)
        kvu_arr[:, :, :, PPOS:2 * HD] = vu[:, :, h0:h0 + NH_L, :].transpose(1, 0, 2, 3)
        kvu_arr[:, :, :, 2 * HD] = counts[None, :, None]
        kvu_arr = np.ascontiguousarray(kvu_arr).astype(NPDT)

        wprojT = np.ascontiguousarray(
            Wproj[:, hs].T.reshape(NH_L, P, H).transpose(1, 0, 2)
        ).astype(NPDT)

        in_maps.append(
            {
                "xT": xT,
                "wqT": wqT,
                "wkvT": wkvT,
                "bq": bq_l,
                "bkv": bkv_l,
                "kvu": kvu_arr,
                "wprojT": wprojT,
                "maskt": mask,
            }
        )
    return in_maps, U


def _ensure_profile_hook():
    """Shim so run_bass_kernel_spmd(trace=True) can capture NTFF profiles."""
    import types

    try:
        import antenv.axon_hooks  # noqa: F401
        return
    except ImportError:
        pass
    try:
        import antenv
        from trn_agent_boot.trn_boot import _ntff_profile_via_ctypes

        m = types.ModuleType("antenv.axon_hooks")
        _hook = [None]
        m.set_axon_ntff_profile_hook = lambda h: _hook.__setitem__(0, h)
        m.get_axon_ntff_profile_hook = lambda: _hook[0]
        sys.modules["antenv.axon_hooks"] = m
        antenv.axon_hooks = m
        m.set_axon_ntff_profile_hook(
            _ntff_profile_via_ctypes("/opt/axon/libaxon_pjrt.so")
        )
    except Exception as e:  # profiling is best-effort
        print(f"profile hook install failed: {e}", file=sys.stderr)


def run(inputs, trace=False):
    """Run on the 8 NeuronCores; returns (output, BassKernelResults)."""
    if trace:
        _ensure_profile_hook()
    in_maps, U = _host_prep(
        inputs["x"], inputs["Wqkv"], inputs["bqkv"], inputs["Wproj"],
        inputs["k_pages"], inputs["v_pages"], inputs["page_table"],
    )
    nc = _get_nc(U)
    res = run_bass_kernel_spmd(
        nc, in_maps, list(range(NCORES)), trace=trace
    )
    acc = np.zeros((H, TOK), np.float64)
    for r in res.results:
        # outT [P, KCH, TOK] -> [H, TOK]: row f*128+p = outT[p, f, :]
        t = np.asarray(r["outT"], np.float64).transpose(1, 0, 2).reshape(H, TOK)
        acc += t
    outf = (acc.T + np.asarray(inputs["bproj"], np.float64)[None, :]).astype(
        np.float32
    )
    return outf.reshape(B, S, H), res


def kernel(**inputs) -> np.ndarray:
    out, _ = run(inputs, trace=False)
    return out


# revision 15
# speedup vs baseline: 1.1142x; 1.1142x over previous
"""Trainium2 Bass kernel for a paged-attention layer (nn_AttentionLayer).

Reference computation (shapes hardcoded from the problem spec):
    x:[4,16,4096] -> qkv = x@Wqkv.T+bqkv -> heads(32,128)
    cached K/V gathered from 48-page pool via page_table[32] (pages of 128)
    full attention (no mask) over 4096 cached + 16 new positions per batch
    out = attn_out @ Wproj.T + bproj            -> [4,16,4096] fp32

Sharding: tensor-parallel over heads. 8 cores x 4 heads. Each core gets its
slice of Wqkv/Wproj/k_pages/v_pages, computes a partial TRANSPOSED output
projection [4096,64] (bf16); partials are summed on the host + bproj.

v3 design:
  - Host dedups the page table (U unique pages); V rows pre-scaled by page
    multiplicity; the per-page block carries a count column so softmax
    numerator+denominator stay exact.
  - All DRAM inputs partition-major; few huge static DMAs issued up-front,
    split across both HWDGE queues (sync+scalar) in consumption order.
  - Every hot matmul keeps a 128-column stationary operand so FWL hides
    LDWEIGHTS: scores (K pages), AV (V pages, flipped -> produces aoT
    directly), proj (Wproj feat-tiles, flipped -> produces outT; host
    un-transposes). Softmax denominators come from count-column lhsT
    matmuls (M=1); normalization is folded into the PSUM->SBUF copy of
    aoT via a gpsimd partition-broadcast of the reciprocal row.
  - QKV runs col-tiled over chunk parity (even k -> PSUM 0:64, odd ->
    64:128) with a partition-shifting DVE copy+add merge.
"""

import os
import sys

for _p in ("/opt/trn_rl_repo", "/root/.axon_site", "/root/.axon_site/_ro/trn_rl_repo"):
    if os.path.isdir(_p) and _p not in sys.path:
        sys.path.append(_p)

import numpy as np
import ml_dtypes

import concourse.bass as bass
import concourse.bacc as bacc
import concourse.mybir as mybir
import concourse.tile as tile
from concourse.masks import make_identity
from concourse.bass_utils import run_bass_kernel_spmd

P = 128
NH = 32           # total heads
NCORES = 8
NH_L = NH // NCORES   # 4 heads per core
HD = 128
B, S = 4, 16
TOK = B * S       # 64
H = 4096
KCH = H // P      # 32 contraction chunks for x@W
PPOS = 128        # page size
PGC = 2 * HD + 1  # per-(page,head) column block: K[128] | V[128] | count
SCALE = 1.0 / float(np.sqrt(np.float32(HD)))

F32 = mybir.dt.float32
BF16 = mybir.dt.bfloat16
NPDT = ml_dtypes.bfloat16

DTYPE_NAME = "bfloat16"   # for test.py's tolerance pick


def _page_groups(U, ng=8):
    """Split U cached pages into up to ng near-equal groups."""
    gs = []
    base = 0
    for i in range(ng):
        n = (U - base + (ng - 1 - i)) // (ng - i)
        if n > 0:
            gs.append((base, base + n))
        base += n
    return gs


def build_nc(U):
    """U = number of unique pages. kvu_sb slots 0..U-1 = cached pages,
    slot U = new-token block (filled on device)."""
    nc = bacc.Bacc("TRN2", target_bir_lowering=False, debug=False)

    xT = nc.dram_tensor("xT", [P, KCH, TOK], BF16, kind="ExternalInput")
    wqT = nc.dram_tensor("wqT", [P, KCH, 512], BF16, kind="ExternalInput")
    wkvT = nc.dram_tensor("wkvT", [P, KCH, 1024], BF16, kind="ExternalInput")
    bq = nc.dram_tensor("bq", [1, 512], BF16, kind="ExternalInput")
    bkv = nc.dram_tensor("bkv", [1, 1024], BF16, kind="ExternalInput")
    kvu = nc.dram_tensor("kvu", [P, U, NH_L, PGC], BF16, kind="ExternalInput")
    wprojT = nc.dram_tensor("wprojT", [P, NH_L, H], BF16, kind="ExternalInput")
    maskt = nc.dram_tensor("maskt", [TOK, TOK], F32, kind="ExternalInput")
    outT = nc.dram_tensor("outT", [P, KCH, TOK], BF16, kind="ExternalOutput")

    with tile.TileContext(nc) as tc:
        _emit(tc, nc, U, xT, wqT, wkvT, bq, bkv, kvu, wprojT, maskt, outT)
    nc.compile()
    return nc


def _emit(tc, nc, U, xT, wqT, wkvT, bq, bkv, kvu, wprojT, maskt, outT,
          dbg=None):
    U1 = U + 1
    Exp = mybir.ActivationFunctionType.Exp
    Add = mybir.AluOpType.add
    Mult = mybir.AluOpType.mult
    groups = _page_groups(U, 4)
    maxg = max(g1 - g0 for (g0, g1) in groups)
    gw = maxg + 1  # attnT buffer width in chunks (newtok uses the extra slot)

    with (
        tc.tile_pool(name="cbuf", bufs=1) as cb,
        tc.tile_pool(name="wpp", bufs=4) as wpp,
        tc.tile_pool(name="big", bufs=2, space="PSUM") as bigp,
        tc.tile_pool(name="scp", bufs=2, space="PSUM") as scp,
        tc.tile_pool(name="avp", bufs=1, space="PSUM") as avp,
        tc.tile_pool(name="dnp", bufs=1, space="PSUM") as dnp,
        tc.tile_pool(name="prp", bufs=2, space="PSUM") as prp,
    ):
        ctr = [0]

        def big_tile(dt=F32):
            ctr[0] += 1
            return bigp.tile([P, 512], dt, tag="big", name=f"big{ctr[0]}")

        def sc_tile():
            ctr[0] += 1
            return scp.tile([P, 512], F32, tag="sc", name=f"sc{ctr[0]}")

        def sc_tile_bf():
            ctr[0] += 1
            return scp.tile([P, 512], BF16, tag="sc", name=f"sc{ctr[0]}")

        # ---- resident SBUF tiles ----
        xT_sb = cb.tile([P, KCH, TOK], BF16, tag="xT")
        wq_sb = cb.tile([P, KCH, 512], BF16, tag="wq")
        wkv_sb = cb.tile([P, KCH, 1024], BF16, tag="wkv")
        kvu_sb = cb.tile([P, U1, NH_L, PGC], BF16, tag="kvu")
        ident = cb.tile([P, P], BF16, tag="ident")
        bq_sb = cb.tile([1, 512], BF16, tag="bq")
        bkv_sb = cb.tile([1, 1024], BF16, tag="bkv")
        ones_sb = cb.tile([1, TOK], BF16, tag="ones")
        mask_sb = cb.tile([TOK, TOK], F32, tag="mask")
        qT_sb = cb.tile([P, NH_L, TOK], BF16, tag="qT")
        aoT_sb = cb.tile([P, NH_L, TOK], BF16, tag="aoT")
        qkv_q = cb.tile([TOK, 512], BF16, tag="qkv_q")
        qkv_kv = cb.tile([TOK, 1024], BF16, tag="qkv_kv")
        attnT = cb.tile([P, NH_L, 2, gw * TOK], BF16, tag="attnT")
        hi_tmp = cb.tile([TOK, 512], F32, tag="hi")
        denr = cb.tile([1, NH_L * TOK], F32, tag="denr")
        rbc = cb.tile([P, NH_L * TOK], F32, tag="rbc")
        obT = cb.tile([P, KCH, TOK], BF16, tag="obT")

        # ---- DMA schedule ----
        # ACT (scalar) issues only 4 early DMAs: its instruction stream
        # carries the exps, and HWDGE ring backpressure on issue would
        # stall them. Sync (idle otherwise) issues everything else; small
        # tensors sit in sync slots 3-5 so completion-count oversync on
        # the first QKV matmul fires early.
        ng = len(groups)
        sc_groups = {0: groups[0]}
        if ng > 2:
            sc_groups[2] = groups[2]
        nc.scalar.dma_start(wq_sb[:, 16:32, :], wqT[:, 16:32, :])
        nc.scalar.dma_start(
            kvu_sb[:, groups[0][0]:groups[0][1], :, :],
            kvu[:, groups[0][0]:groups[0][1], :, :])
        nc.scalar.dma_start(wkv_sb[:, 0:8, :], wkvT[:, 0:8, :])
        if ng > 2:
            nc.scalar.dma_start(
                kvu_sb[:, groups[2][0]:groups[2][1], :, :],
                kvu[:, groups[2][0]:groups[2][1], :, :])

        nc.sync.dma_start(wq_sb[:, 0:16, :], wqT[:, 0:16, :])
        nc.sync.dma_start(xT_sb[:], xT[:])
        nc.sync.dma_start(bq_sb[:], bq[:])
        nc.sync.dma_start(bkv_sb[:], bkv[:])
        nc.sync.dma_start(mask_sb[:], maskt[:])
        sync_groups = [g for gi, g in enumerate(groups) if gi not in sc_groups]
        if sync_groups:
            g0, g1 = sync_groups[0]
            nc.sync.dma_start(kvu_sb[:, g0:g1, :, :], kvu[:, g0:g1, :, :])
        nc.sync.dma_start(wkv_sb[:, 8:16, :], wkvT[:, 8:16, :])
        for (g0, g1) in sync_groups[1:]:
            nc.sync.dma_start(kvu_sb[:, g0:g1, :, :], kvu[:, g0:g1, :, :])
        nc.sync.dma_start(wkv_sb[:, 16:24, :], wkvT[:, 16:24, :])
        nc.sync.dma_start(wkv_sb[:, 24:32, :], wkvT[:, 24:32, :])
        wp_tiles = []
        for sidx in range(4):
            wp = wpp.tile([P, NH_L, 1024], BF16, tag="wp", name=f"wp{sidx}")
            nc.sync.dma_start(wp[:], wprojT[:, :, sidx * 1024:(sidx + 1) * 1024])
            wp_tiles.append(wp)

        # ---- setup ----
        make_identity(nc, ident[:])
        nc.gpsimd.memset(ones_sb[:], 1.0)
        # new-token slot U: zero K pad + V rows + count, count=1 valid rows
        nc.gpsimd.memset(kvu_sb[:, U, :, :], 0.0)
        nc.gpsimd.memset(kvu_sb[:TOK, U, :, 2 * HD:], 1.0)

        # warm the PE HAM clock gate while the first DMAs land
        ps_warm = big_tile()
        for _ in range(30):
            nc.tensor.matmul(
                ps_warm[:, :P], lhsT=ident[:], rhs=ident[:],
                start=True, stop=True,
            )

        # ---- QKV (q part): col-tiled over chunk parity ----
        ps_q = big_tile()
        for k in range(KCH):
            par = k % 2
            nc.tensor.matmul(
                ps_q[64 * par:64 * (par + 1), :],
                lhsT=xT_sb[:, k, :],
                rhs=wq_sb[:, k, :],
                start=(k < 2),
                stop=(k == KCH - 1),
                tile_position=(0, 64 * par),
            )
        nc.tensor.matmul(
            ps_q[64:128, :], lhsT=ones_sb[:], rhs=bq_sb[:],
            start=False, stop=True, tile_position=(0, 64),
        )
        nc.vector.tensor_copy(hi_tmp[:], ps_q[64:128, :])
        nc.vector.tensor_tensor(
            out=qkv_q[:], in0=ps_q[0:64, :], in1=hi_tmp[:], op=Add
        )
        for hl in range(NH_L):
            ps_t = big_tile(BF16)
            nc.tensor.transpose(
                ps_t[:, :TOK], qkv_q[:, hl * HD:(hl + 1) * HD],
                ident[:TOK, :TOK],
            )
            nc.vector.tensor_copy(qT_sb[:, hl, :], ps_t[:, :TOK])

        # ---- attention (kv-QKV batches interleaved below) ----
        # ps_av[:, hl*64:(hl+1)*64] accumulates unnormalized aoT per head
        ps_av = avp.tile([P, NH_L * TOK], F32, tag="av")
        ps_den = dnp.tile([1, NH_L * TOK], F32, tag="den")

        def attn_block(gi, g0, g1, is_new):
            par = gi % 2
            n = g1 - g0
            # scores + exp, all heads
            for hl in range(NH_L):
                ps_sc = sc_tile()
                for c in range(g0, g1):
                    nc.tensor.matmul(
                        ps_sc[:, (c - g0) * TOK:(c - g0 + 1) * TOK],
                        lhsT=kvu_sb[:, c, hl, 0:PPOS],
                        rhs=qT_sb[:, hl, :],
                        start=True, stop=True,
                    )
                if is_new:  # block-diagonal batch mask on raw scores
                    nc.vector.tensor_tensor(
                        out=ps_sc[:TOK, 0:TOK], in0=ps_sc[:TOK, 0:TOK],
                        in1=mask_sb[:], op=Add,
                    )
                nc.scalar.activation(
                    attnT[:, hl, par, 0:n * TOK], ps_sc[:, 0:n * TOK],
                    Exp, scale=SCALE,
                )
            # AV (flipped: V stationary, 128-col FWL) + denominator
            for hl in range(NH_L):
                for c in range(g0, g1):
                    a_sl = attnT[:, hl, par, (c - g0) * TOK:(c - g0 + 1) * TOK]
                    nc.tensor.matmul(
                        ps_av[:, hl * TOK:(hl + 1) * TOK],
                        lhsT=kvu_sb[:, c, hl, PPOS:PPOS + HD],
                        rhs=a_sl,
                        start=(gi == 0 and hl == 0 and c == g0),
                        stop=(is_new and hl == NH_L - 1 and c == g1 - 1),
                        skip_group_check=True,
                    )
                for c in range(g0, g1):
                    a_sl = attnT[:, hl, par, (c - g0) * TOK:(c - g0 + 1) * TOK]
                    nc.tensor.matmul(
                        ps_den[:, hl * TOK:(hl + 1) * TOK],
                        lhsT=kvu_sb[:, c, hl, 2 * HD:PGC],
                        rhs=a_sl,
                        start=(gi == 0 and hl == 0 and c == g0),
                        stop=(is_new and hl == NH_L - 1 and c == g1 - 1),
                        skip_group_check=True,
                    )

        ps_kv = [big_tile(), big_tile()]

        def kv_batch(k0, k1):
            for k in range(k0, k1):
                par = k % 2
                for j in range(2):
                    nc.tensor.matmul(
                        ps_kv[j][64 * par:64 * (par + 1), :],
                        lhsT=xT_sb[:, k, :],
                        rhs=wkv_sb[:, k, 512 * j:512 * (j + 1)],
                        start=(k < 2),
                        stop=(k == KCH - 1),
                        tile_position=(0, 64 * par),
                    )

        natt = 0
        nkv = 0

        def att_next():
            nonlocal natt
            g0, g1 = groups[natt]
            attn_block(natt, g0, g1, False)
            natt += 1

        def kv_next():
            nonlocal nkv
            kv_batch(nkv * 8, (nkv + 1) * 8)
            nkv += 1

        plan = ["a", "a", "k", "k", "a", "a", "k", "k"]
        for step in plan:
            if step == "a":
                if natt < len(groups):
                    att_next()
            else:
                kv_next()
        # kv bias + merge + new-token slot fill
        for j in range(2):
            nc.tensor.matmul(
                ps_kv[j][64:128, :], lhsT=ones_sb[:],
                rhs=bkv_sb[:, 512 * j:512 * (j + 1)],
                start=False, stop=True, tile_position=(0, 64),
            )
        for j in range(2):
            nc.vector.tensor_copy(hi_tmp[:], ps_kv[j][64:128, :])
            nc.vector.tensor_tensor(
                out=qkv_kv[:, 512 * j:512 * (j + 1)],
                in0=ps_kv[j][0:64, :], in1=hi_tmp[:], op=Add,
            )
        for hl in range(NH_L):
            ps_t = sc_tile_bf()
            nc.tensor.transpose(
                ps_t[:, :TOK], qkv_kv[:, hl * 256:hl * 256 + HD],
                ident[:TOK, :TOK],
            )
            nc.vector.tensor_copy(kvu_sb[:, U, hl, 0:TOK], ps_t[:, :TOK])
            nc.vector.tensor_copy(
                kvu_sb[:TOK, U, hl, PPOS:PPOS + HD],
                qkv_kv[:, hl * 256 + HD:hl * 256 + 2 * HD],
            )
        attn_block(natt, U, U1, True)

        # ---- normalize: recip row -> broadcast -> fold into aoT copy ----
        nc.vector.reciprocal(denr[:], ps_den[:])
        nc.gpsimd.partition_broadcast(rbc[:], denr[:])
        for hl in range(NH_L):
            nc.vector.tensor_tensor(
                out=aoT_sb[:, hl, :],
                in0=ps_av[:, hl * TOK:(hl + 1) * TOK],
                in1=rbc[:, hl * TOK:(hl + 1) * TOK],
                op=Mult,
            )

        # ---- output projection (flipped: Wproj tiles stationary) ----
        for sidx in range(4):
            wp = wp_tiles[sidx]
            ps_o = prp.tile([P, 512], F32, tag="pr", name=f"pr{sidx}")
            for fi in range(8):
                for hl in range(NH_L):
                    nc.tensor.matmul(
                        ps_o[:, fi * TOK:(fi + 1) * TOK],
                        lhsT=wp[:, hl, fi * HD:(fi + 1) * HD],
                        rhs=aoT_sb[:, hl, :],
                        start=(hl == 0), stop=(hl == NH_L - 1),
                    )
            nc.vector.tensor_copy(
                obT[:, sidx * 8:(sidx + 1) * 8, :], ps_o[:]
            )
            if sidx % 2 == 1:
                half = sidx // 2
                nc.sync.dma_start(
                    outT[:, half * 16:(half + 1) * 16, :],
                    obT[:, half * 16:(half + 1) * 16, :],
                )

        if dbg is not None:
            nc.sync.dma_start(dbg["d_qT"][:], qT_sb[:])
            nc.sync.dma_start(dbg["d_aoT"][:], aoT_sb[:])
            nc.sync.dma_start(dbg["d_denr"][:], denr[:])
            nc.sync.dma_start(dbg["d_kvuU"][:], kvu_sb[:, U, :, :])
            nc.sync.dma_start(dbg["d_qkvkv"][:], qkv_kv[:])


_NC_CACHE = {}


def _get_nc(U):
    if U not in _NC_CACHE:
        _NC_CACHE[U] = build_nc(U)
    return _NC_CACHE[U]


def _host_prep(x, Wqkv, bqkv, Wproj, k_pages, v_pages, page_table):
    """Build the 8 per-core input maps (numpy, partition-major layouts)."""
    x = np.asarray(x, np.float32)
    Wqkv = np.asarray(Wqkv, np.float32)
    bqkv = np.asarray(bqkv, np.float32)
    Wproj = np.asarray(Wproj, np.float32)
    k_pages = np.asarray(k_pages, np.float32)
    v_pages = np.asarray(v_pages, np.float32)
    pt = np.asarray(page_table, np.int64)

    upages, counts = np.unique(pt, return_counts=True)
    U = len(upages)

    xT = np.ascontiguousarray(
        x.reshape(TOK, H).T.reshape(KCH, P, TOK).transpose(1, 0, 2)
    ).astype(NPDT)

    mask = np.full((TOK, TOK), -1e30, np.float32)
    for b in range(B):
        mask[b * S:(b + 1) * S, b * S:(b + 1) * S] = 0.0

    Wq, Wk, Wv = Wqkv[:H], Wqkv[H:2 * H], Wqkv[2 * H:]
    bqf, bkf, bvf = bqkv[:H], bqkv[H:2 * H], bqkv[2 * H:]

    # gather unique pages once for all cores: [U, PPOS, NH, HD]
    ku = k_pages[upages]
    vu = v_pages[upages] * counts[:, None, None, None].astype(np.float32)

    in_maps = []
    for c in range(NCORES):
        h0 = c * NH_L
        hs = slice(h0 * HD, (h0 + NH_L) * HD)

        wqT = np.ascontiguousarray(
            Wq[hs].T.reshape(KCH, P, NH_L * HD).transpose(1, 0, 2)
        ).astype(NPDT)
        wkv_rows = np.empty((NH_L * 2 * HD, H), np.float32)
        for h in range(NH_L):
            r = slice((h0 + h) * HD, (h0 + h + 1) * HD)
            wkv_rows[h * 256:h * 256 + HD] = Wk[r]
            wkv_rows[h * 256 + HD:h * 256 + 2 * HD] = Wv[r]
        wkvT = np.ascontiguousarray(
            wkv_rows.T.reshape(KCH, P, 1024).transpose(1, 0, 2)
        ).astype(NPDT)

        bq_l = bqf[hs].reshape(1, 512).astype(NPDT)
        bkv_l = np.empty((1, 1024), np.float32)
        for h in range(NH_L):
            bkv_l[0, h * 256:h * 256 + HD] = bkf[(h0 + h) * HD:(h0 + h + 1) * HD]
            bkv_l[0, h * 256 + HD:h * 256 + 2 * HD] = (
                bvf[(h0 + h) * HD:(h0 + h + 1) * HD]
            )
        bkv_l = bkv_l.astype(NPDT)

        kvu_arr = np.zeros((P, U, NH_L, PGC), np.float32)
        # K block: [p=hd, u, h, c=pos]
        kvu_arr[:, :, :, 0:PPOS] = ku[:, :, h0:h0 + NH_L, :].transpose(3, 0, 2, 1)
        # V block: [p=pos, u, h, c=hd] (count-scaled)
        kvu_arr[:, :, :, PPOS:2 * HD] = vu[:, :, h0:h0 + NH_L, :].transpose(1, 0, 2, 3)
        kvu_arr[:, :, :, 2 * HD] = counts[None, :, None]
        kvu_arr = np.ascontiguousarray(kvu_arr).astype(NPDT)

        wprojT = np.ascontiguousarray(
            Wproj[:, hs].T.reshape(NH_L, P, H).transpose(1, 0, 2)
        ).astype(NPDT)

        in_maps.append(
            {
                "xT": xT,
                "wqT": wqT,
                "wkvT": wkvT,
                "bq": bq_l,
                "bkv": bkv_l,
                "kvu": kvu_arr,
                "wprojT": wprojT,
                "maskt": mask,
            }
        )
    return in_maps, U


def _ensure_profile_hook():
    """Shim so run_bass_kernel_spmd(trace=True) can capture NTFF profiles."""
    import types

    try:
        import antenv.axon_hooks  # noqa: F401
        return
    except ImportError:
        pass
    try:
        import antenv
        from trn_agent_boot.trn_boot import _ntff_profile_via_ctypes

        m = types.ModuleType("antenv.axon_hooks")
        _hook = [None]
        m.set_axon_ntff_profile_hook = lambda h: _hook.__setitem__(0, h)
        m.get_axon_ntff_profile_hook = lambda: _hook[0]
        sys.modules["antenv.axon_hooks"] = m
        antenv.axon_hooks = m
        m.set_axon_ntff_profile_hook(
            _ntff_profile_via_ctypes("/opt/axon/libaxon_pjrt.so")
        )
    except Exception as e:  # profiling is best-effort
        print(f"profile hook install failed: {e}", file=sys.stderr)


def run(inputs, trace=False):
    """Run on the 8 NeuronCores; returns (output, BassKernelResults)."""
    if trace:
        _ensure_profile_hook()
    in_maps, U = _host_prep(
        inputs["x"], inputs["Wqkv"], inputs["bqkv"], inputs["Wproj"],
        inputs["k_pages"], inputs["v_pages"], inputs["page_table"],
    )
    nc = _get_nc(U)
    res = run_bass_kernel_spmd(
        nc, in_maps, list(range(NCORES)), trace=trace
    )
    acc = np.zeros((H, TOK), np.float64)
    for r in res.results:
        # outT [P, KCH, TOK] -> [H, TOK]: row f*128+p = outT[p, f, :]
        t = np.asarray(r["outT"], np.float64).transpose(1, 0, 2).reshape(H, TOK)
        acc += t
    outf = (acc.T + np.asarray(inputs["bproj"], np.float64)[None, :]).astype(
        np.float32
    )
    return outf.reshape(B, S, H), res


def kernel(**inputs) -> np.ndarray:
    out, _ = run(inputs, trace=False)
    return out
